# revision 30
# baseline (speedup 1.0000x reference)
"""Differential cross-attention head on 8 Trainium2 NeuronCores.

Sharding: data-parallel over batch (4) x sequence-parallel over Tq (2) = 8 cores.
Each core computes out[b, h*1024:(h+1)*1024, :] for (b, h) = divmod(core, 2).

Per-core math in "transposed" orientation (no on-chip transposes; host
transposes the output back):
  - qT = Wq^T @ xT            [D, 1024]
  - kT = Wk^T @ encT          [D, Tk]
  - v  = encT^T @ Wv          [Tk, D]
  - s^T = k @ q^T             [Tk, Tq] scores transposed; s1|s2 packed into one
                              [128,1024] PSUM tile via PE row-group tiling
  - e^T = exp(s^T/8)          ScalarE, PSUM->SBUF, bf16
  - A^T += v_chunk^T @ e^T    accumulated in PSUM
  - row-sums: DVE chain accumulation + ones-matmul partition reduce
Normalization out = A1/r1 - lam*A2/r2 and final transpose happen on the host.

Schedule: both q-groups interleaved per Tk chunk so ScalarE (exp, the ~33us
floor) never waits for a phase change. PSUM budget (8 banks): 2 rotating score
tiles (4) + pv0 [A1|A2 of g0] (2) + pv1 [A1 of g1] (1) + a dedicated projection
bank (1). Group-1's A2 half is accumulated later from stored exp tiles, after
the last projection releases the projection bank - that is what frees the bank
that lets k/v projections software-pipeline into the previous Tk-group's
attention stream without stalling ScalarE.

b_k is never applied: adding k-bias shifts every score of a query column by
the same constant (s[k,q] += bk . q[:,q]), and softmax over Tk is invariant
to per-column shifts. b_q does change the weights and is applied when nonzero.

DMA: all transfers have 4-8KB contiguous rows (DMA engines serve queues
packet-by-packet, so per-queue bandwidth scales with packet size), split
across the two HWDGE queues (Sync, Scalar) + GpSimd SWDGE, priority-ordered.
"""

import sys
from contextlib import ExitStack

import numpy as np

_TRN_REPO = "/opt/trn_rl_repo"
if _TRN_REPO not in sys.path:
    sys.path.insert(0, _TRN_REPO)

import ml_dtypes

import concourse.bass as bass
import concourse.tile as tile
from concourse import mybir
from concourse.bass import ds, ts

F32 = mybir.dt.float32
BF16 = mybir.dt.bfloat16

E = 1024
D = 128
B = 4
TQ = 2048
TK = 2048
NCORES = 8
TQL = B * TQ // NCORES   # 1024 query rows per core
EC = E // 128            # 8 contraction chunks for projections
NG = TQL // 512          # 2 query groups of 512
TKG = TK // 512          # 4 Tk groups
KC = TK // 128           # 16 Tk chunks
SCALE = 0.125            # 1/sqrt(64)

NP_BF16 = ml_dtypes.bfloat16


def _build(nc: bass.Bass, with_qbias: bool, with_vbias: bool):
    xT = nc.dram_tensor("xT", [NG, 128, EC, 512], BF16,
                        kind="ExternalInput").ap()
    encT = nc.dram_tensor("encT", [TKG, 128, EC, 512], BF16,
                          kind="ExternalInput").ap()
    wpack = nc.dram_tensor("wpack", [128, 3, EC, D], BF16,
                           kind="ExternalInput").ap()
    bpack = nc.dram_tensor("bpack", [128, 2], F32, kind="ExternalInput").ap()
    bv = nc.dram_tensor("bv", [D], F32, kind="ExternalInput").ap()
    pvd = nc.dram_tensor("pvd", [128, NG * 1024], BF16,
                         kind="ExternalOutput").ap()
    rd = nc.dram_tensor("rd", [NG, 2, 512], F32, kind="ExternalOutput").ap()

    Exp = mybir.ActivationFunctionType.Exp

    with tile.TileContext(nc) as tc, ExitStack() as ctx:
        const = ctx.enter_context(tc.tile_pool(name="const", bufs=1))
        xpool = ctx.enter_context(tc.tile_pool(name="xpool", bufs=1))
        encpool = ctx.enter_context(tc.tile_pool(name="encpool", bufs=1))
        proj = ctx.enter_context(tc.tile_pool(name="proj", bufs=1))
        epool = ctx.enter_context(tc.tile_pool(name="epool", bufs=6))
        g1pool = ctx.enter_context(tc.tile_pool(name="g1pool", bufs=KC))
        rpool = ctx.enter_context(tc.tile_pool(name="rpool", bufs=4))
        outp = ctx.enter_context(tc.tile_pool(name="outp", bufs=2))
        psS = ctx.enter_context(tc.tile_pool(name="psS", bufs=2, space="PSUM"))
        psPV = ctx.enter_context(tc.tile_pool(name="psPV", bufs=1, space="PSUM"))
        psP = ctx.enter_context(tc.tile_pool(name="psP", bufs=1, space="PSUM"))

        # ---- input DMAs: 8KB contiguous rows everywhere (queue bandwidth is
        # packet-proportional); big tensors partition-split [0:64]/[64:128]
        # so three queues can be loaded to match each piece's need-time ----
        w3_sb = const.tile([128, 3, EC, D], BF16, tag="w3")
        xstage = xpool.tile([128, NG, EC, 512], BF16, tag="xstage")
        enc_sb = encpool.tile([128, TKG, EC, 512], BF16, tag="enc")
        P0, P1 = slice(0, 64), slice(64, 128)

        # sync HWDGE: wq+wk then x-g0 (full width, 8KB rows)
        nc.sync.dma_start(out=w3_sb[:, 0:2], in_=wpack[:, 0:2])   # wq, wk
        if with_qbias:
            b_sb = const.tile([128, 2], F32, tag="b")
            nc.sync.dma_start(out=b_sb, in_=bpack)
        nc.sync.dma_start(out=xstage[:, 0], in_=xT[0])

        # scalar HWDGE: enc-tg0, wv, enc-tg1, x-g1, enc-tg2 (need-time order)
        nc.scalar.dma_start(out=enc_sb[:, 0], in_=encT[0])
        nc.scalar.dma_start(out=w3_sb[:, 2:3], in_=wpack[:, 2:3])  # wv
        if with_vbias:
            bv_sb = const.tile([1, D], F32, tag="bv")
            nc.scalar.dma_start(out=bv_sb,
                                in_=bv.rearrange("(o d) -> o d", o=1))
        nc.scalar.dma_start(out=enc_sb[:, 1], in_=encT[1])
        nc.scalar.dma_start(out=xstage[:, 1], in_=xT[1])
        nc.scalar.dma_start(out=enc_sb[:, 2], in_=encT[2])

        # gpsimd SWDGE: enc-tg3 (needed last; tolerates slow completion sem)
        nc.gpsimd.dma_start(out=enc_sb[:, 3], in_=encT[3])
        _ = P0, P1

        if with_vbias:
            ones_row_f32 = const.tile([1, 128], F32, tag="ones_row_f32")
            nc.vector.memset(ones_row_f32, 1.0)
        ones_col = const.tile([128, 1], BF16, tag="ones_col")
        nc.vector.memset(ones_col, 1.0)


        qT_sb = proj.tile([128, TQL], BF16, tag="qT")
        kT_sb = proj.tile([128, TK], BF16, tag="kT")
        v_sb = proj.tile([128, KC, D], BF16, tag="v")

        def qproj(g, pool):
            qp = pool.tile([128, 512] if pool is psP else [128, 1024],
                           F32, tag="ps_p" if pool is psP else "ps_s",
                           name=f"qp{g}")
            for c in range(EC):
                nc.tensor.matmul(qp[:, 0:512], lhsT=w3_sb[:, 0, c],
                                 rhs=xstage[:, g, c],
                                 start=(c == 0), stop=(c == EC - 1))
            if with_qbias:
                nc.vector.tensor_scalar_add(qT_sb[:, ts(g, 512)], qp[:, 0:512],
                                            b_sb[:, 0:1])
            else:
                nc.vector.tensor_copy(qT_sb[:, ts(g, 512)], qp[:, 0:512])

        # ---- projection steps for one tk-group, as small closures that the
        # main loop interleaves between attention units ----
        kp_box = [None]
        vp_box = [None]

        def kp_mm(tg, c0, c1):
            def step():
                if c0 == 0:
                    kp_box[0] = psP.tile([128, 512], F32, tag="ps_p",
                                         name=f"kp{tg}")
                for c in range(c0, c1):
                    nc.tensor.matmul(kp_box[0], lhsT=w3_sb[:, 1, c],
                                     rhs=enc_sb[:, tg, c],
                                     start=(c == 0), stop=(c == EC - 1))
            return step

        def kp_drain(tg):
            def step():
                # b_k is dropped: softmax over Tk is invariant to the
                # per-query-column shift it induces.
                nc.vector.tensor_copy(kT_sb[:, ts(tg, 512)], kp_box[0])
            return step

        def vp_block(tg, t):
            # one [128,128] v block: 8 matmuls + immediate per-block drain so
            # downstream PV units unblock as early as possible
            def step():
                if t == 0:
                    vp_box[0] = psP.tile([128, 512], F32, tag="ps_p",
                                         name=f"vp{tg}")
                tk = tg * 4 + t
                if with_vbias:
                    nc.tensor.matmul(vp_box[0][:, ts(t, 128)],
                                     lhsT=ones_row_f32, rhs=bv_sb,
                                     start=True, stop=False,
                                     skip_group_check=True)
                for c in range(EC):
                    nc.tensor.matmul(vp_box[0][:, ts(t, 128)],
                                     lhsT=enc_sb[:, tg, c, ts(t, 128)],
                                     rhs=w3_sb[:, 2, c],
                                     start=(not with_vbias and c == 0),
                                     stop=(c == EC - 1),
                                     skip_group_check=True)
                nc.vector.tensor_copy(v_sb[:, tk, :], vp_box[0][:, ts(t, 128)])
            return step

        # ---- attention ----
        pv0 = [None]
        pv1 = [None]
        a2_box = [None]
        g1e = [None] * KC
        racc = [[rpool.tile([128, 1024], BF16, tag="racc", name=f"racc{g}{p}")
                 for p in range(2)] for g in range(NG)]

        def emit_pv(g, k_glob, e12):
            if g == 0:
                if k_glob == 0:
                    pv0[0] = psPV.tile([128, 1024], F32, tag="pv0", name="pv0")
                for h in range(2):
                    nc.tensor.matmul(pv0[0][:, ts(h, 512)],
                                     lhsT=v_sb[:, k_glob, :],
                                     rhs=e12[:, ts(h, 512)],
                                     start=(k_glob == 0),
                                     stop=(k_glob == KC - 1),
                                     skip_group_check=True)
            else:
                if k_glob == 0:
                    pv1[0] = psPV.tile([128, 512], F32, tag="pv1", name="pv1")
                nc.tensor.matmul(pv1[0],
                                 lhsT=v_sb[:, k_glob, :],
                                 rhs=e12[:, 0:512],
                                 start=(k_glob == 0), stop=(k_glob == KC - 1),
                                 skip_group_check=True)

        g0e = {}

        def attention_unit(g, k_glob, defer_pv=False):
            s12 = psS.tile([128, 1024], F32, tag="ps_s", name="s12")
            nc.tensor.matmul(s12[:, 0:512],
                             lhsT=kT_sb[0:64, ts(k_glob, 128)],
                             rhs=qT_sb[0:64, ts(g, 512)],
                             start=True, stop=True, tile_position=(0, 0))
            nc.tensor.matmul(s12[:, 512:1024],
                             lhsT=kT_sb[64:128, ts(k_glob, 128)],
                             rhs=qT_sb[64:128, ts(g, 512)],
                             start=True, stop=True, tile_position=(64, 0))
            if g == 0:
                e12 = epool.tile([128, 1024], BF16, tag="e", name="e12")
            else:
                e12 = g1pool.tile([128, 1024], BF16, tag="e1",
                                  name=f"e1_{k_glob}")
                g1e[k_glob] = e12
            nc.scalar.activation(e12, s12, Exp, scale=SCALE)
            if defer_pv:
                g0e[k_glob] = e12
            else:
                emit_pv(g, k_glob, e12)
            par = k_glob % 2
            if k_glob < 2:
                nc.vector.tensor_copy(racc[g][par], e12)
            else:
                nc.vector.tensor_add(racc[g][par], racc[g][par], e12)

        def pv_flush(*ks):
            def step():
                for k in ks:
                    emit_pv(0, k, g0e.pop(k))
            return step

        def a2_mm(k):
            # group-1 A2 = sum_k v^T exp(s2), from stored exp tiles, using
            # the projection bank after the last projection released it
            if k == 0:
                a2_box[0] = psP.tile([128, 512], F32, tag="ps_p", name="a2")
            nc.tensor.matmul(a2_box[0], lhsT=v_sb[:, k, :],
                             rhs=g1e[k][:, 512:1024],
                             start=(k == 0), stop=(k == KC - 1),
                             skip_group_check=True)

        def attention_tail(g):
            # both racc chains reduced directly in PSUM accumulation - no
            # DVE merge on the critical tail
            r12p = psS.tile([65, 512], F32, tag="ps_s", name="r12p")
            for p in range(2):
                nc.tensor.matmul(r12p[0:1, :], lhsT=ones_col,
                                 rhs=racc[g][p][:, 0:512],
                                 start=(p == 0), stop=(p == 1),
                                 skip_group_check=True)
                nc.tensor.matmul(r12p[64:65, :], lhsT=ones_col,
                                 rhs=racc[g][p][:, 512:1024],
                                 start=(p == 0), stop=(p == 1),
                                 skip_group_check=True)
            r_sb = outp.tile([65, 512], F32, tag="r_sb", name=f"r_sb{g}")
            nc.vector.tensor_copy(r_sb, r12p)
            nc.sync.dma_start(out=rd[g, 0], in_=r_sb[0:1, :])
            nc.sync.dma_start(out=rd[g, 1], in_=r_sb[64:65, :])
            pv_sb = outp.tile([128, 1024], BF16, tag="pv_sb", name=f"pv_sb{g}")
            if g == 0:
                nc.scalar.copy(pv_sb, pv0[0])
            else:
                nc.scalar.copy(pv_sb[:, 0:512], pv1[0])
                nc.scalar.copy(pv_sb[:, 512:1024], a2_box[0])
            eng = nc.sync if g == 0 else nc.scalar
            eng.dma_start(out=pvd[:, ds(g * 1024, 1024)], in_=pv_sb)

        # ---- schedule: group 1 lags LAG chunks behind group 0 so its late
        # x / qT arrival never blocks the in-order PE stream; projections for
        # tk-group N spread over stream positions 4(N-1)..4(N-1)+3; A2
        # catch-up once the projection bank is released ----
        LAG = 4

        # prologue: k-proj(tg0) + qT(g0) only - the minimum for the first
        # exp; v-proj(tg0) is deferred into the first stream positions with
        # PV(g0, 0..5) flushed as v blocks drain
        kp_mm(0, 0, 4)()
        kp_mm(0, 4, 8)()
        kp_drain(0)()
        qproj(0, psS)

        POS = {
            0:  [vp_block(0, 0), vp_block(0, 1)],
            1:  [vp_block(0, 2), vp_block(0, 3), pv_flush(0, 1)],
            2:  [kp_mm(1, 0, 4), kp_mm(1, 4, 8), pv_flush(2)],
            3:  [kp_drain(1), lambda: qproj(1, psP), pv_flush(3)],
            4:  [vp_block(1, 0), vp_block(1, 1), pv_flush(4)],
            5:  [vp_block(1, 2), vp_block(1, 3), pv_flush(5)],
            6:  [kp_mm(2, 0, 4), kp_mm(2, 4, 8)],
            7:  [kp_drain(2), vp_block(2, 0)],
            8:  [vp_block(2, 1), vp_block(2, 2)],
            9:  [vp_block(2, 3), kp_mm(3, 0, 4)],
            10: [kp_mm(3, 4, 8), kp_drain(3)],
            11: [vp_block(3, 0), vp_block(3, 1)],
            12: [vp_block(3, 2), vp_block(3, 3)],
            13: [lambda: (a2_mm(0), a2_mm(1), a2_mm(2))],
            14: [lambda: (a2_mm(3), a2_mm(4), a2_mm(5))],
            15: [lambda: (a2_mm(6), a2_mm(7), a2_mm(8))],
        }
        for p in range(KC):
            attention_unit(0, p, defer_pv=(p <= 5))
            if p >= LAG:
                attention_unit(1, p - LAG)
            for step in POS.get(p, []):
                step()

        # group-0 tail overlaps group-1's remaining units
        attention_tail(0)
        tail_a2 = {12: (9, 10), 13: (11, 12), 14: (13,), 15: (14, 15)}
        for k in range(KC - LAG, KC):
            attention_unit(1, k)
            for a in tail_a2[k]:
                a2_mm(a)
        attention_tail(1)

    return nc


_nc_cache = {}


def _make_bass(with_qbias: bool, with_vbias: bool):
    from concourse import bacc

    nc = bacc.Bacc("TRN2", target_bir_lowering=False, debug=False)
    _build(nc, with_qbias, with_vbias)
    nc.compile()
    return nc


def _pack_x(a):
    """[T, E] -> [T//512, 128, EC, 512] bf16 (partition-major per group)."""
    t = a.shape[0]
    at = np.ascontiguousarray(a.T.astype(NP_BF16))       # [E, T]
    at = at.reshape(EC, 128, t // 512, 512)              # [c, p, g, 512]
    return np.ascontiguousarray(at.transpose(2, 1, 0, 3))  # [g, p, c, 512]


def _pack_w(W_q, W_k, W_v):
    """3x [E, D] -> [128, 3, EC, D] bf16."""
    w = np.stack([np.asarray(w, np.float32) for w in (W_q, W_k, W_v)])
    w = w.astype(NP_BF16).reshape(3, EC, 128, D)
    return np.ascontiguousarray(w.transpose(2, 0, 1, 3))


def kernel(x, encoder_out, W_q, b_q, W_k, b_k, W_v, b_v,
           lambda_q1, lambda_k1, lambda_q2, lambda_k2, lambda_init):
    from concourse import bass_utils

    x = np.asarray(x, np.float32)
    encoder_out = np.asarray(encoder_out, np.float32)
    wpack = _pack_w(W_q, W_k, W_v)
    bpack = np.ascontiguousarray(
        np.stack([np.asarray(b_q, np.float32),
                  np.asarray(b_k, np.float32)], axis=1))  # [128, 2]
    b_v = np.asarray(b_v, np.float32)

    lam = np.float32(
        np.exp(np.float32(np.asarray(lambda_q1, np.float32)
                          @ np.asarray(lambda_k1, np.float32)))
        - np.exp(np.float32(np.asarray(lambda_q2, np.float32)
                            @ np.asarray(lambda_k2, np.float32)))
        + np.float32(np.asarray(lambda_init, np.float32))
    )

    with_qbias = bool(np.any(np.asarray(b_q, np.float32)))
    with_vbias = bool(np.any(b_v))
    key = (with_qbias, with_vbias)
    if key not in _nc_cache:
        _nc_cache[key] = _make_bass(*key)
    nc = _nc_cache[key]

    encTs = [_pack_x(encoder_out[b]) for b in range(B)]  # [TKG,128,EC,512]
    in_maps = []
    for c in range(NCORES):
        b, h = divmod(c, 2)
        xTs = _pack_x(x[b, h * TQL:(h + 1) * TQL, :])
        in_maps.append({
            "xT": xTs, "encT": encTs[b],
            "wpack": wpack, "bpack": bpack, "bv": b_v,
        })

    res = bass_utils.run_bass_kernel_spmd(nc, in_maps, core_ids=list(range(NCORES)))
    kernel.last_result = res

    out = np.empty((B, TQ, D), np.float32)
    for c in range(NCORES):
        b, h = divmod(c, 2)
        pvd = np.asarray(res.results[c]["pvd"], np.float32)  # [D, NG*1024]
        rd = np.asarray(res.results[c]["rd"], np.float32)    # [NG, 2, 512]
        for g in range(NG):
            A = pvd[:, g * 1024:(g + 1) * 1024]
            A1, A2 = A[:, 0:512], A[:, 512:1024]
            r1, r2 = rd[g, 0], rd[g, 1]
            o = A1 / r1 - lam * (A2 / r2)    # [D, 512]
            q0 = h * TQL + g * 512
            out[b, q0:q0 + 512, :] = o.T
    return out


# revision 33
# speedup vs baseline: 1.1922x; 1.1922x over previous
"""Differential cross-attention head on 8 Trainium2 NeuronCores.

Sharding: data-parallel over batch (4) x sequence-parallel over Tq (2) = 8 cores.
Each core computes out[b, h*1024:(h+1)*1024, :] for (b, h) = divmod(core, 2).

Per-core math in "transposed" orientation (no on-chip transposes; host
transposes the output back):
  - qT = Wq^T @ xT            [D, 1024]
  - kT = Wk^T @ encT          [D, Tk]
  - v  = encT^T @ Wv          [Tk, D]
  - s^T = k @ q^T             [Tk, Tq] scores transposed; s1|s2 packed into one
                              [128,1024] PSUM tile via PE row-group tiling
  - e^T = exp(s^T/8)          ScalarE, PSUM->SBUF, bf16
  - A^T += v_chunk^T @ e^T    accumulated in PSUM
  - row-sums: DVE chain accumulation + ones-matmul partition reduce
Normalization out = A1/r1 - lam*A2/r2 and final transpose happen on the host.

Schedule: both q-groups interleaved per Tk chunk so ScalarE (exp, the ~33us
floor) never waits for a phase change. PSUM budget (8 banks): 2 rotating score
tiles (4) + pv0 [A1|A2 of g0] (2) + pv1 [A1 of g1] (1) + a dedicated projection
bank (1). Group-1's A2 half is accumulated later from stored exp tiles, after
the last projection releases the projection bank - that is what frees the bank
that lets k/v projections software-pipeline into the previous Tk-group's
attention stream without stalling ScalarE.

b_k is never applied: adding k-bias shifts every score of a query column by
the same constant (s[k,q] += bk . q[:,q]), and softmax over Tk is invariant
to per-column shifts. b_q does change the weights and is applied when nonzero.

DMA: all transfers have 4-8KB contiguous rows (DMA engines serve queues
packet-by-packet, so per-queue bandwidth scales with packet size), split
across the two HWDGE queues (Sync, Scalar) + GpSimd SWDGE, priority-ordered.
"""

import sys
from contextlib import ExitStack

import numpy as np

_TRN_REPO = "/opt/trn_rl_repo"
if _TRN_REPO not in sys.path:
    sys.path.insert(0, _TRN_REPO)

import ml_dtypes

import concourse.bass as bass
import concourse.tile as tile
from concourse import mybir
from concourse.bass import ds, ts

F32 = mybir.dt.float32
BF16 = mybir.dt.bfloat16

E = 1024
D = 128
B = 4
TQ = 2048
TK = 2048
NCORES = 8
TQL = B * TQ // NCORES   # 1024 query rows per core
EC = E // 128            # 8 contraction chunks for projections
NG = TQL // 512          # 2 query groups of 512
TKG = TK // 512          # 4 Tk groups
KC = TK // 128           # 16 Tk chunks
SCALE = 0.125            # 1/sqrt(64)

NP_BF16 = ml_dtypes.bfloat16


def _build(nc: bass.Bass, with_qbias: bool, with_vbias: bool):
    xT = nc.dram_tensor("xT", [NG, 128, EC, 512], BF16,
                        kind="ExternalInput").ap()
    encT = nc.dram_tensor("encT", [TKG, 128, EC, 512], BF16,
                          kind="ExternalInput").ap()
    wpack = nc.dram_tensor("wpack", [128, 3, EC, D], BF16,
                           kind="ExternalInput").ap()
    bpack = nc.dram_tensor("bpack", [128, 2], F32, kind="ExternalInput").ap()
    bv = nc.dram_tensor("bv", [D], F32, kind="ExternalInput").ap()
    pvd = nc.dram_tensor("pvd", [128, NG * 1024], BF16,
                         kind="ExternalOutput").ap()
    rd = nc.dram_tensor("rd", [NG, 2, 512], F32, kind="ExternalOutput").ap()

    Exp = mybir.ActivationFunctionType.Exp

    with tile.TileContext(nc) as tc, ExitStack() as ctx:
        const = ctx.enter_context(tc.tile_pool(name="const", bufs=1))
        xpool = ctx.enter_context(tc.tile_pool(name="xpool", bufs=1))
        encpool = ctx.enter_context(tc.tile_pool(name="encpool", bufs=1))
        proj = ctx.enter_context(tc.tile_pool(name="proj", bufs=1))
        epool = ctx.enter_context(tc.tile_pool(name="epool", bufs=6))
        g1pool = ctx.enter_context(tc.tile_pool(name="g1pool", bufs=KC))
        rpool = ctx.enter_context(tc.tile_pool(name="rpool", bufs=4))
        outp = ctx.enter_context(tc.tile_pool(name="outp", bufs=2))
        psS = ctx.enter_context(tc.tile_pool(name="psS", bufs=2, space="PSUM"))
        psPV = ctx.enter_context(tc.tile_pool(name="psPV", bufs=1, space="PSUM"))
        psP = ctx.enter_context(tc.tile_pool(name="psP", bufs=1, space="PSUM"))

        # ---- input DMAs: 8KB contiguous rows everywhere (queue bandwidth is
        # packet-proportional); big tensors partition-split [0:64]/[64:128]
        # so three queues can be loaded to match each piece's need-time ----
        w3_sb = const.tile([128, 3, EC, D], BF16, tag="w3")
        xstage = xpool.tile([128, NG, EC, 512], BF16, tag="xstage")
        enc_sb = encpool.tile([128, TKG, EC, 512], BF16, tag="enc")
        P0, P1 = slice(0, 64), slice(64, 128)

        # sync HWDGE: wq+wk then x-g0 (full width, 8KB rows)
        nc.sync.dma_start(out=w3_sb[:, 0:2], in_=wpack[:, 0:2])   # wq, wk
        if with_qbias:
            b_sb = const.tile([128, 2], F32, tag="b")
            nc.sync.dma_start(out=b_sb, in_=bpack)
        nc.sync.dma_start(out=xstage[:, 0], in_=xT[0])

        # scalar HWDGE: enc-tg0, wv, enc-tg1, x-g1, enc-tg2 (need-time order)
        nc.scalar.dma_start(out=enc_sb[:, 0], in_=encT[0])
        nc.scalar.dma_start(out=w3_sb[:, 2:3], in_=wpack[:, 2:3])  # wv
        if with_vbias:
            bv_sb = const.tile([1, D], F32, tag="bv")
            nc.scalar.dma_start(out=bv_sb,
                                in_=bv.rearrange("(o d) -> o d", o=1))
        nc.scalar.dma_start(out=enc_sb[:, 1], in_=encT[1])
        nc.scalar.dma_start(out=xstage[:, 1], in_=xT[1])
        nc.scalar.dma_start(out=enc_sb[:, 2], in_=encT[2])

        # gpsimd SWDGE: enc-tg3 (needed last; tolerates slow completion sem)
        nc.gpsimd.dma_start(out=enc_sb[:, 3], in_=encT[3])
        _ = P0, P1

        if with_vbias:
            ones_row_f32 = const.tile([1, 128], F32, tag="ones_row_f32")
            nc.vector.memset(ones_row_f32, 1.0)
        ones_col = const.tile([128, 1], BF16, tag="ones_col")
        nc.vector.memset(ones_col, 1.0)


        qT_sb = proj.tile([128, TQL], BF16, tag="qT")
        kT_sb = proj.tile([128, TK], BF16, tag="kT")
        v_sb = proj.tile([128, KC, D], BF16, tag="v")

        def qproj(g, pool):
            qp = pool.tile([128, 512] if pool is psP else [128, 1024],
                           F32, tag="ps_p" if pool is psP else "ps_s",
                           name=f"qp{g}")
            for c in range(EC):
                nc.tensor.matmul(qp[:, 0:512], lhsT=w3_sb[:, 0, c],
                                 rhs=xstage[:, g, c],
                                 start=(c == 0), stop=(c == EC - 1))
            if with_qbias:
                nc.vector.tensor_scalar_add(qT_sb[:, ts(g, 512)], qp[:, 0:512],
                                            b_sb[:, 0:1])
            else:
                nc.vector.tensor_copy(qT_sb[:, ts(g, 512)], qp[:, 0:512])

        # ---- projection steps for one tk-group, as small closures that the
        # main loop interleaves between attention units ----
        kp_box = [None]
        vp_box = [None]

        def kp_mm(tg, c0, c1):
            def step():
                if c0 == 0:
                    kp_box[0] = psP.tile([128, 512], F32, tag="ps_p",
                                         name=f"kp{tg}")
                for c in range(c0, c1):
                    nc.tensor.matmul(kp_box[0], lhsT=w3_sb[:, 1, c],
                                     rhs=enc_sb[:, tg, c],
                                     start=(c == 0), stop=(c == EC - 1))
            return step

        def kp_drain(tg):
            def step():
                # b_k is dropped: softmax over Tk is invariant to the
                # per-query-column shift it induces.
                nc.vector.tensor_copy(kT_sb[:, ts(tg, 512)], kp_box[0])
            return step

        def vp_mm(tg, t):
            def step():
                if t == 0:
                    vp_box[0] = psP.tile([128, 512], F32, tag="ps_p",
                                         name=f"vp{tg}")
                tk = tg * 4 + t
                if with_vbias:
                    nc.tensor.matmul(vp_box[0][:, ts(t, 128)],
                                     lhsT=ones_row_f32, rhs=bv_sb,
                                     start=True, stop=False,
                                     skip_group_check=True)
                for c in range(EC):
                    nc.tensor.matmul(vp_box[0][:, ts(t, 128)],
                                     lhsT=enc_sb[:, tg, c, ts(t, 128)],
                                     rhs=w3_sb[:, 2, c],
                                     start=(not with_vbias and c == 0),
                                     stop=(c == EC - 1),
                                     skip_group_check=True)
            return step

        def vp_drain(tg):
            def step():
                nc.vector.tensor_copy(
                    v_sb[:, tg * 4:(tg + 1) * 4, :]
                    .rearrange("p t d -> p (t d)"),
                    vp_box[0])
            return step

        def proj_steps(tg):
            return [kp_mm(tg, 0, 4), kp_mm(tg, 4, 8), kp_drain(tg),
                    vp_mm(tg, 0), vp_mm(tg, 1), vp_mm(tg, 2), vp_mm(tg, 3),
                    vp_drain(tg)]

        # ---- attention ----
        pv0 = [None]
        pv1 = [None]
        a2_box = [None]
        g1e = [None] * KC
        racc = [[rpool.tile([128, 1024], BF16, tag="racc", name=f"racc{g}{p}")
                 for p in range(2)] for g in range(NG)]

        def emit_pv(g, k_glob, e12):
            if g == 0:
                if k_glob == 0:
                    pv0[0] = psPV.tile([128, 1024], F32, tag="pv0", name="pv0")
                for h in range(2):
                    nc.tensor.matmul(pv0[0][:, ts(h, 512)],
                                     lhsT=v_sb[:, k_glob, :],
                                     rhs=e12[:, ts(h, 512)],
                                     start=(k_glob == 0),
                                     stop=(k_glob == KC - 1),
                                     skip_group_check=True)
            else:
                if k_glob == 0:
                    pv1[0] = psPV.tile([128, 512], F32, tag="pv1", name="pv1")
                nc.tensor.matmul(pv1[0],
                                 lhsT=v_sb[:, k_glob, :],
                                 rhs=e12[:, 0:512],
                                 start=(k_glob == 0), stop=(k_glob == KC - 1),
                                 skip_group_check=True)

        g0e = {}

        def attention_unit(g, k_glob, defer_pv=False):
            s12 = psS.tile([128, 1024], F32, tag="ps_s", name="s12")
            nc.tensor.matmul(s12[:, 0:512],
                             lhsT=kT_sb[0:64, ts(k_glob, 128)],
                             rhs=qT_sb[0:64, ts(g, 512)],
                             start=True, stop=True, tile_position=(0, 0))
            nc.tensor.matmul(s12[:, 512:1024],
                             lhsT=kT_sb[64:128, ts(k_glob, 128)],
                             rhs=qT_sb[64:128, ts(g, 512)],
                             start=True, stop=True, tile_position=(64, 0))
            if g == 0:
                e12 = epool.tile([128, 1024], BF16, tag="e", name="e12")
            else:
                e12 = g1pool.tile([128, 1024], BF16, tag="e1",
                                  name=f"e1_{k_glob}")
                g1e[k_glob] = e12
            nc.scalar.activation(e12, s12, Exp, scale=SCALE)
            if defer_pv:
                g0e[k_glob] = e12
            else:
                emit_pv(g, k_glob, e12)
            par = k_glob % 2
            if k_glob < 2:
                nc.vector.tensor_copy(racc[g][par], e12)
            else:
                nc.vector.tensor_add(racc[g][par], racc[g][par], e12)

        def pv_flush(*ks):
            def step():
                for k in ks:
                    emit_pv(0, k, g0e.pop(k))
            return step

        def a2_mm(k):
            # group-1 A2 = sum_k v^T exp(s2), from stored exp tiles, using
            # the projection bank after the last projection released it
            if k == 0:
                a2_box[0] = psP.tile([128, 512], F32, tag="ps_p", name="a2")
            nc.tensor.matmul(a2_box[0], lhsT=v_sb[:, k, :],
                             rhs=g1e[k][:, 512:1024],
                             start=(k == 0), stop=(k == KC - 1),
                             skip_group_check=True)

        def attention_tail(g):
            # both racc chains reduced directly in PSUM accumulation - no
            # DVE merge on the critical tail
            r12p = psS.tile([65, 512], F32, tag="ps_s", name="r12p")
            for p in range(2):
                nc.tensor.matmul(r12p[0:1, :], lhsT=ones_col,
                                 rhs=racc[g][p][:, 0:512],
                                 start=(p == 0), stop=(p == 1),
                                 skip_group_check=True)
                nc.tensor.matmul(r12p[64:65, :], lhsT=ones_col,
                                 rhs=racc[g][p][:, 512:1024],
                                 start=(p == 0), stop=(p == 1),
                                 skip_group_check=True)
            r_sb = outp.tile([65, 512], F32, tag="r_sb", name=f"r_sb{g}")
            nc.vector.tensor_copy(r_sb, r12p)
            nc.sync.dma_start(out=rd[g, 0], in_=r_sb[0:1, :])
            nc.sync.dma_start(out=rd[g, 1], in_=r_sb[64:65, :])
            pv_sb = outp.tile([128, 1024], BF16, tag="pv_sb", name=f"pv_sb{g}")
            if g == 0:
                nc.scalar.copy(pv_sb, pv0[0])
            else:
                nc.scalar.copy(pv_sb[:, 0:512], pv1[0])
                nc.scalar.copy(pv_sb[:, 512:1024], a2_box[0])
            eng = nc.sync if g == 0 else nc.scalar
            eng.dma_start(out=pvd[:, ds(g * 1024, 1024)], in_=pv_sb)

        # ---- schedule: group 1 lags LAG chunks behind group 0 so its late
        # x / qT arrival never blocks the in-order PE stream; projections for
        # tk-group N spread over stream positions 4(N-1)..4(N-1)+3; A2
        # catch-up once the projection bank is released ----
        LAG = 4

        # prologue in expected data-arrival order: enc-tg0/wv, then x-g0
        for step in proj_steps(0):
            step()
        qproj(0, psS)

        pos_steps = {}
        s1 = proj_steps(1)
        s1.append(lambda: qproj(1, psP))
        pos_steps[0] = s1[0:2]
        pos_steps[1] = s1[2:4]
        pos_steps[2] = s1[4:6]
        pos_steps[3] = s1[6:9]
        s2 = proj_steps(2)
        pos_steps[4] = s2[0:2]
        pos_steps[5] = s2[2:4]
        pos_steps[6] = s2[4:6]
        pos_steps[7] = s2[6:8]
        s3 = proj_steps(3)
        pos_steps[8] = s3[0:2]
        pos_steps[9] = s3[2:4]
        pos_steps[10] = s3[4:6]
        pos_steps[11] = s3[6:8]
        pos_steps[12] = [lambda k=k: a2_mm(k) for k in (0, 1, 2)]
        pos_steps[13] = [lambda k=k: a2_mm(k) for k in (3, 4, 5)]
        pos_steps[14] = [lambda k=k: a2_mm(k) for k in (6, 7)]
        pos_steps[15] = [lambda k=k: a2_mm(k) for k in (8, 9)]

        for p in range(KC):
            attention_unit(0, p)
            if p >= LAG:
                attention_unit(1, p - LAG)
            for step in pos_steps.get(p, []):
                step()

        # group-0 tail overlaps group-1's remaining units
        attention_tail(0)
        for k in range(KC - LAG, KC):
            attention_unit(1, k)
            a2_mm(k - 2)
            if k == KC - 1:
                a2_mm(k - 1)
                a2_mm(k)
        attention_tail(1)

    return nc


_nc_cache = {}


def _make_bass(with_qbias: bool, with_vbias: bool):
    from concourse import bacc

    nc = bacc.Bacc("TRN2", target_bir_lowering=False, debug=False)
    _build(nc, with_qbias, with_vbias)
    nc.compile()
    return nc


def _pack_x(a):
    """[T, E] -> [T//512, 128, EC, 512] bf16 (partition-major per group)."""
    t = a.shape[0]
    at = np.ascontiguousarray(a.T.astype(NP_BF16))       # [E, T]
    at = at.reshape(EC, 128, t // 512, 512)              # [c, p, g, 512]
    return np.ascontiguousarray(at.transpose(2, 1, 0, 3))  # [g, p, c, 512]


def _pack_w(W_q, W_k, W_v):
    """3x [E, D] -> [128, 3, EC, D] bf16."""
    w = np.stack([np.asarray(w, np.float32) for w in (W_q, W_k, W_v)])
    w = w.astype(NP_BF16).reshape(3, EC, 128, D)
    return np.ascontiguousarray(w.transpose(2, 0, 1, 3))


def kernel(x, encoder_out, W_q, b_q, W_k, b_k, W_v, b_v,
           lambda_q1, lambda_k1, lambda_q2, lambda_k2, lambda_init):
    from concourse import bass_utils

    x = np.asarray(x, np.float32)
    encoder_out = np.asarray(encoder_out, np.float32)
    wpack = _pack_w(W_q, W_k, W_v)
    bpack = np.ascontiguousarray(
        np.stack([np.asarray(b_q, np.float32),
                  np.asarray(b_k, np.float32)], axis=1))  # [128, 2]
    b_v = np.asarray(b_v, np.float32)

    lam = np.float32(
        np.exp(np.float32(np.asarray(lambda_q1, np.float32)
                          @ np.asarray(lambda_k1, np.float32)))
        - np.exp(np.float32(np.asarray(lambda_q2, np.float32)
                            @ np.asarray(lambda_k2, np.float32)))
        + np.float32(np.asarray(lambda_init, np.float32))
    )

    with_qbias = bool(np.any(np.asarray(b_q, np.float32)))
    with_vbias = bool(np.any(b_v))
    key = (with_qbias, with_vbias)
    if key not in _nc_cache:
        _nc_cache[key] = _make_bass(*key)
    nc = _nc_cache[key]

    encTs = [_pack_x(encoder_out[b]) for b in range(B)]  # [TKG,128,EC,512]
    in_maps = []
    for c in range(NCORES):
        b, h = divmod(c, 2)
        xTs = _pack_x(x[b, h * TQL:(h + 1) * TQL, :])
        in_maps.append({
            "xT": xTs, "encT": encTs[b],
            "wpack": wpack, "bpack": bpack, "bv": b_v,
        })

    res = bass_utils.run_bass_kernel_spmd(nc, in_maps, core_ids=list(range(NCORES)))
    kernel.last_result = res

    out = np.empty((B, TQ, D), np.float32)
    for c in range(NCORES):
        b, h = divmod(c, 2)
        pvd = np.asarray(res.results[c]["pvd"], np.float32)  # [D, NG*1024]
        rd = np.asarray(res.results[c]["rd"], np.float32)    # [NG, 2, 512]
        for g in range(NG):
            A = pvd[:, g * 1024:(g + 1) * 1024]
            A1, A2 = A[:, 0:512], A[:, 512:1024]
            r1, r2 = rd[g, 0], rd[g, 1]
            o = A1 / r1 - lam * (A2 / r2)    # [D, 512]
            q0 = h * TQL + g * 512
            out[b, q0:q0 + 512, :] = o.T
    return out


# revision 35
# speedup vs baseline: 1.1987x; 1.0055x over previous
"""Differential cross-attention head on 8 Trainium2 NeuronCores.

Sharding: data-parallel over batch (4) x sequence-parallel over Tq (2) = 8 cores.
Each core computes out[b, h*1024:(h+1)*1024, :] for (b, h) = divmod(core, 2).

Per-core math in "transposed" orientation (no on-chip transposes; host
transposes the output back):
  - qT = Wq^T @ xT            [D, 1024]
  - kT = Wk^T @ encT          [D, Tk]
  - v  = encT^T @ Wv          [Tk, D]
  - s^T = k @ q^T             [Tk, Tq] scores transposed; s1|s2 packed into one
                              [128,1024] PSUM tile via PE row-group tiling
  - e^T = exp(s^T/8)          ScalarE, PSUM->SBUF, bf16
  - A^T += v_chunk^T @ e^T    accumulated in PSUM
  - row-sums: DVE chain accumulation + ones-matmul partition reduce
Normalization out = A1/r1 - lam*A2/r2 and final transpose happen on the host.

Schedule: both q-groups interleaved per Tk chunk so ScalarE (exp, the ~33us
floor) never waits for a phase change. PSUM budget (8 banks): 2 rotating score
tiles (4) + pv0 [A1|A2 of g0] (2) + pv1 [A1 of g1] (1) + a dedicated projection
bank (1). Group-1's A2 half is accumulated later from stored exp tiles, after
the last projection releases the projection bank - that is what frees the bank
that lets k/v projections software-pipeline into the previous Tk-group's
attention stream without stalling ScalarE.

b_k is never applied: adding k-bias shifts every score of a query column by
the same constant (s[k,q] += bk . q[:,q]), and softmax over Tk is invariant
to per-column shifts. b_q does change the weights and is applied when nonzero.

DMA: all transfers have 4-8KB contiguous rows (DMA engines serve queues
packet-by-packet, so per-queue bandwidth scales with packet size), split
across the two HWDGE queues (Sync, Scalar) + GpSimd SWDGE, priority-ordered.
"""

import sys
from contextlib import ExitStack

import numpy as np

_TRN_REPO = "/opt/trn_rl_repo"
if _TRN_REPO not in sys.path:
    sys.path.insert(0, _TRN_REPO)

import ml_dtypes

import concourse.bass as bass
import concourse.tile as tile
from concourse import mybir
from concourse.bass import ds, ts

F32 = mybir.dt.float32
BF16 = mybir.dt.bfloat16

E = 1024
D = 128
B = 4
TQ = 2048
TK = 2048
NCORES = 8
TQL = B * TQ // NCORES   # 1024 query rows per core
EC = E // 128            # 8 contraction chunks for projections
NG = TQL // 512          # 2 query groups of 512
TKG = TK // 512          # 4 Tk groups
KC = TK // 128           # 16 Tk chunks
SCALE = 0.125            # 1/sqrt(64)

NP_BF16 = ml_dtypes.bfloat16


def _build(nc: bass.Bass, with_qbias: bool, with_vbias: bool):
    xT = nc.dram_tensor("xT", [NG, 128, EC, 512], BF16,
                        kind="ExternalInput").ap()
    encT = nc.dram_tensor("encT", [TKG, 128, EC, 512], BF16,
                          kind="ExternalInput").ap()
    wpack = nc.dram_tensor("wpack", [128, 3, EC, D], BF16,
                           kind="ExternalInput").ap()
    bpack = nc.dram_tensor("bpack", [128, 2], F32, kind="ExternalInput").ap()
    bv = nc.dram_tensor("bv", [D], F32, kind="ExternalInput").ap()
    pvd = nc.dram_tensor("pvd", [128, NG * 1024], BF16,
                         kind="ExternalOutput").ap()
    rd = nc.dram_tensor("rd", [NG, 2, 512], F32, kind="ExternalOutput").ap()

    Exp = mybir.ActivationFunctionType.Exp

    with tile.TileContext(nc) as tc, ExitStack() as ctx:
        const = ctx.enter_context(tc.tile_pool(name="const", bufs=1))
        xpool = ctx.enter_context(tc.tile_pool(name="xpool", bufs=1))
        encpool = ctx.enter_context(tc.tile_pool(name="encpool", bufs=1))
        proj = ctx.enter_context(tc.tile_pool(name="proj", bufs=1))
        epool = ctx.enter_context(tc.tile_pool(name="epool", bufs=6))
        g1pool = ctx.enter_context(tc.tile_pool(name="g1pool", bufs=KC))
        rpool = ctx.enter_context(tc.tile_pool(name="rpool", bufs=4))
        outp = ctx.enter_context(tc.tile_pool(name="outp", bufs=2))
        psS = ctx.enter_context(tc.tile_pool(name="psS", bufs=2, space="PSUM"))
        psPV = ctx.enter_context(tc.tile_pool(name="psPV", bufs=1, space="PSUM"))
        psP = ctx.enter_context(tc.tile_pool(name="psP", bufs=1, space="PSUM"))

        # ---- input DMAs: 8KB contiguous rows everywhere (queue bandwidth is
        # packet-proportional); big tensors partition-split [0:64]/[64:128]
        # so three queues can be loaded to match each piece's need-time ----
        w3_sb = const.tile([128, 3, EC, D], BF16, tag="w3")
        xstage = xpool.tile([128, NG, EC, 512], BF16, tag="xstage")
        enc_sb = encpool.tile([128, TKG, EC, 512], BF16, tag="enc")
        P0, P1 = slice(0, 64), slice(64, 128)

        # sync HWDGE: wq+wk then x-g0 (full width, 8KB rows)
        nc.sync.dma_start(out=w3_sb[:, 0:2], in_=wpack[:, 0:2])   # wq, wk
        if with_qbias:
            b_sb = const.tile([128, 2], F32, tag="b")
            nc.sync.dma_start(out=b_sb, in_=bpack)
        nc.sync.dma_start(out=xstage[:, 0], in_=xT[0])

        # scalar HWDGE: enc-tg0, wv, enc-tg1, x-g1, enc-tg2 (need-time order)
        nc.scalar.dma_start(out=enc_sb[:, 0], in_=encT[0])
        nc.scalar.dma_start(out=w3_sb[:, 2:3], in_=wpack[:, 2:3])  # wv
        if with_vbias:
            bv_sb = const.tile([1, D], F32, tag="bv")
            nc.scalar.dma_start(out=bv_sb,
                                in_=bv.rearrange("(o d) -> o d", o=1))
        nc.scalar.dma_start(out=enc_sb[:, 1], in_=encT[1])
        nc.scalar.dma_start(out=xstage[:, 1], in_=xT[1])
        nc.scalar.dma_start(out=enc_sb[:, 2], in_=encT[2])

        # gpsimd SWDGE: enc-tg3 (needed last; tolerates slow completion sem)
        nc.gpsimd.dma_start(out=enc_sb[:, 3], in_=encT[3])
        _ = P0, P1

        if with_vbias:
            ones_row_f32 = const.tile([1, 128], F32, tag="ones_row_f32")
            nc.vector.memset(ones_row_f32, 1.0)
        ones_col = const.tile([128, 1], BF16, tag="ones_col")
        nc.vector.memset(ones_col, 1.0)


        qT_sb = proj.tile([128, TQL], BF16, tag="qT")
        kT_sb = proj.tile([128, TK], BF16, tag="kT")
        v_sb = proj.tile([128, KC, D], BF16, tag="v")

        def qproj(g, pool):
            qp = pool.tile([128, 512] if pool is psP else [128, 1024],
                           F32, tag="ps_p" if pool is psP else "ps_s",
                           name=f"qp{g}")
            for c in range(EC):
                nc.tensor.matmul(qp[:, 0:512], lhsT=w3_sb[:, 0, c],
                                 rhs=xstage[:, g, c],
                                 start=(c == 0), stop=(c == EC - 1))
            if with_qbias:
                nc.vector.tensor_scalar_add(qT_sb[:, ts(g, 512)], qp[:, 0:512],
                                            b_sb[:, 0:1])
            else:
                nc.vector.tensor_copy(qT_sb[:, ts(g, 512)], qp[:, 0:512])

        # ---- projection steps for one tk-group, as small closures that the
        # main loop interleaves between attention units ----
        kp_box = [None]
        vp_box = [None]

        def kp_mm(tg, c0, c1):
            def step():
                if c0 == 0:
                    kp_box[0] = psP.tile([128, 512], F32, tag="ps_p",
                                         name=f"kp{tg}")
                for c in range(c0, c1):
                    nc.tensor.matmul(kp_box[0], lhsT=w3_sb[:, 1, c],
                                     rhs=enc_sb[:, tg, c],
                                     start=(c == 0), stop=(c == EC - 1))
            return step

        def kp_drain(tg):
            def step():
                # b_k is dropped: softmax over Tk is invariant to the
                # per-query-column shift it induces.
                nc.vector.tensor_copy(kT_sb[:, ts(tg, 512)], kp_box[0])
            return step

        def vp_mm(tg, t):
            def step():
                if t == 0:
                    vp_box[0] = psP.tile([128, 512], F32, tag="ps_p",
                                         name=f"vp{tg}")
                tk = tg * 4 + t
                if with_vbias:
                    nc.tensor.matmul(vp_box[0][:, ts(t, 128)],
                                     lhsT=ones_row_f32, rhs=bv_sb,
                                     start=True, stop=False,
                                     skip_group_check=True)
                for c in range(EC):
                    nc.tensor.matmul(vp_box[0][:, ts(t, 128)],
                                     lhsT=enc_sb[:, tg, c, ts(t, 128)],
                                     rhs=w3_sb[:, 2, c],
                                     start=(not with_vbias and c == 0),
                                     stop=(c == EC - 1),
                                     skip_group_check=True)
            return step

        def vp_drain(tg):
            def step():
                nc.vector.tensor_copy(
                    v_sb[:, tg * 4:(tg + 1) * 4, :]
                    .rearrange("p t d -> p (t d)"),
                    vp_box[0])
            return step

        def proj_steps(tg):
            return [kp_mm(tg, 0, 4), kp_mm(tg, 4, 8), kp_drain(tg),
                    vp_mm(tg, 0), vp_mm(tg, 1), vp_mm(tg, 2), vp_mm(tg, 3),
                    vp_drain(tg)]

        # ---- attention ----
        pv0 = [None]
        pv1 = [None]
        a2_box = [None]
        g1e = [None] * KC
        racc = [[rpool.tile([128, 1024], BF16, tag="racc", name=f"racc{g}{p}")
                 for p in range(2)] for g in range(NG)]

        def emit_pv(g, k_glob, e12):
            if g == 0:
                if k_glob == 0:
                    pv0[0] = psPV.tile([128, 1024], F32, tag="pv0", name="pv0")
                for h in range(2):
                    nc.tensor.matmul(pv0[0][:, ts(h, 512)],
                                     lhsT=v_sb[:, k_glob, :],
                                     rhs=e12[:, ts(h, 512)],
                                     start=(k_glob == 0),
                                     stop=(k_glob == KC - 1),
                                     skip_group_check=True)
            else:
                if k_glob == 0:
                    pv1[0] = psPV.tile([128, 512], F32, tag="pv1", name="pv1")
                nc.tensor.matmul(pv1[0],
                                 lhsT=v_sb[:, k_glob, :],
                                 rhs=e12[:, 0:512],
                                 start=(k_glob == 0), stop=(k_glob == KC - 1),
                                 skip_group_check=True)

        g0e = {}

        def attention_unit(g, k_glob, defer_pv=False):
            s12 = psS.tile([128, 1024], F32, tag="ps_s", name="s12")
            nc.tensor.matmul(s12[:, 0:512],
                             lhsT=kT_sb[0:64, ts(k_glob, 128)],
                             rhs=qT_sb[0:64, ts(g, 512)],
                             start=True, stop=True, tile_position=(0, 0))
            nc.tensor.matmul(s12[:, 512:1024],
                             lhsT=kT_sb[64:128, ts(k_glob, 128)],
                             rhs=qT_sb[64:128, ts(g, 512)],
                             start=True, stop=True, tile_position=(64, 0))
            if g == 0:
                e12 = epool.tile([128, 1024], BF16, tag="e", name="e12")
            else:
                e12 = g1pool.tile([128, 1024], BF16, tag="e1",
                                  name=f"e1_{k_glob}")
                g1e[k_glob] = e12
            nc.scalar.activation(e12, s12, Exp, scale=SCALE)
            if defer_pv:
                g0e[k_glob] = e12
            else:
                emit_pv(g, k_glob, e12)
            par = k_glob % 2
            if k_glob < 2:
                nc.vector.tensor_copy(racc[g][par], e12)
            else:
                nc.vector.tensor_add(racc[g][par], racc[g][par], e12)

        def pv_flush(*ks):
            def step():
                for k in ks:
                    emit_pv(0, k, g0e.pop(k))
            return step

        def a2_mm(k):
            # group-1 A2 = sum_k v^T exp(s2), from stored exp tiles, using
            # the projection bank after the last projection released it
            if k == 0:
                a2_box[0] = psP.tile([128, 512], F32, tag="ps_p", name="a2")
            nc.tensor.matmul(a2_box[0], lhsT=v_sb[:, k, :],
                             rhs=g1e[k][:, 512:1024],
                             start=(k == 0), stop=(k == KC - 1),
                             skip_group_check=True)

        def attention_tail(g):
            # both racc chains reduced directly in PSUM accumulation - no
            # DVE merge on the critical tail
            r12p = psS.tile([65, 512], F32, tag="ps_s", name="r12p")
            for p in range(2):
                nc.tensor.matmul(r12p[0:1, :], lhsT=ones_col,
                                 rhs=racc[g][p][:, 0:512],
                                 start=(p == 0), stop=(p == 1),
                                 skip_group_check=True)
                nc.tensor.matmul(r12p[64:65, :], lhsT=ones_col,
                                 rhs=racc[g][p][:, 512:1024],
                                 start=(p == 0), stop=(p == 1),
                                 skip_group_check=True)
            r_sb = outp.tile([65, 512], F32, tag="r_sb", name=f"r_sb{g}")
            nc.vector.tensor_copy(r_sb, r12p)
            nc.sync.dma_start(out=rd[g, 0], in_=r_sb[0:1, :])
            nc.sync.dma_start(out=rd[g, 1], in_=r_sb[64:65, :])
            pv_sb = outp.tile([128, 1024], BF16, tag="pv_sb", name=f"pv_sb{g}")
            if g == 0:
                nc.scalar.copy(pv_sb, pv0[0])
            else:
                nc.scalar.copy(pv_sb[:, 0:512], pv1[0])
                nc.scalar.copy(pv_sb[:, 512:1024], a2_box[0])
            eng = nc.sync if g == 0 else nc.scalar
            eng.dma_start(out=pvd[:, ds(g * 1024, 1024)], in_=pv_sb)

        # ---- schedule: group 1 lags LAG chunks behind group 0 so its late
        # x / qT arrival never blocks the in-order PE stream; projections for
        # tk-group N spread over stream positions 4(N-1)..4(N-1)+3; A2
        # catch-up once the projection bank is released ----
        LAG = 4

        # prologue: k-proj(tg0) and qT(g0) only, then the first two attention
        # units fire (PV deferred); v-proj(tg0) + tg1's projections follow
        # while the exp stream is already running
        ps0 = proj_steps(0)
        for step in ps0[0:3]:
            step()
        qproj(0, psS)
        attention_unit(0, 0, defer_pv=True)
        attention_unit(0, 1, defer_pv=True)
        for step in ps0[3:8]:
            step()
        pv_flush(0, 1)()

        pos_steps = {}
        s1 = proj_steps(1)
        s1.append(lambda: qproj(1, psP))
        for step in s1[0:4]:
            step()
        pos_steps[2] = s1[4:6]
        pos_steps[3] = s1[6:9]
        s2 = proj_steps(2)
        pos_steps[4] = s2[0:2]
        pos_steps[5] = s2[2:4]
        pos_steps[6] = s2[4:6]
        pos_steps[7] = s2[6:8]
        s3 = proj_steps(3)
        pos_steps[8] = s3[0:2]
        pos_steps[9] = s3[2:4]
        pos_steps[10] = s3[4:6]
        pos_steps[11] = s3[6:8]
        pos_steps[12] = [lambda k=k: a2_mm(k) for k in (0, 1, 2)]
        pos_steps[13] = [lambda k=k: a2_mm(k) for k in (3, 4, 5)]
        pos_steps[14] = [lambda k=k: a2_mm(k) for k in (6, 7)]
        pos_steps[15] = [lambda k=k: a2_mm(k) for k in (8, 9)]

        for p in range(2, KC):
            attention_unit(0, p)
            if p >= LAG:
                attention_unit(1, p - LAG)
            for step in pos_steps.get(p, []):
                step()

        # group-0 tail overlaps group-1's remaining units
        attention_tail(0)
        for k in range(KC - LAG, KC):
            attention_unit(1, k)
            a2_mm(k - 2)
            if k == KC - 1:
                a2_mm(k - 1)
                a2_mm(k)
        attention_tail(1)

    return nc


_nc_cache = {}


def _make_bass(with_qbias: bool, with_vbias: bool):
    from concourse import bacc

    nc = bacc.Bacc("TRN2", target_bir_lowering=False, debug=False)
    _build(nc, with_qbias, with_vbias)
    nc.compile()
    return nc


def _pack_x(a):
    """[T, E] -> [T//512, 128, EC, 512] bf16 (partition-major per group)."""
    t = a.shape[0]
    at = np.ascontiguousarray(a.T.astype(NP_BF16))       # [E, T]
    at = at.reshape(EC, 128, t // 512, 512)              # [c, p, g, 512]
    return np.ascontiguousarray(at.transpose(2, 1, 0, 3))  # [g, p, c, 512]


def _pack_w(W_q, W_k, W_v):
    """3x [E, D] -> [128, 3, EC, D] bf16."""
    w = np.stack([np.asarray(w, np.float32) for w in (W_q, W_k, W_v)])
    w = w.astype(NP_BF16).reshape(3, EC, 128, D)
    return np.ascontiguousarray(w.transpose(2, 0, 1, 3))


def kernel(x, encoder_out, W_q, b_q, W_k, b_k, W_v, b_v,
           lambda_q1, lambda_k1, lambda_q2, lambda_k2, lambda_init):
    from concourse import bass_utils

    x = np.asarray(x, np.float32)
    encoder_out = np.asarray(encoder_out, np.float32)
    wpack = _pack_w(W_q, W_k, W_v)
    bpack = np.ascontiguousarray(
        np.stack([np.asarray(b_q, np.float32),
                  np.asarray(b_k, np.float32)], axis=1))  # [128, 2]
    b_v = np.asarray(b_v, np.float32)

    lam = np.float32(
        np.exp(np.float32(np.asarray(lambda_q1, np.float32)
                          @ np.asarray(lambda_k1, np.float32)))
        - np.exp(np.float32(np.asarray(lambda_q2, np.float32)
                            @ np.asarray(lambda_k2, np.float32)))
        + np.float32(np.asarray(lambda_init, np.float32))
    )

    with_qbias = bool(np.any(np.asarray(b_q, np.float32)))
    with_vbias = bool(np.any(b_v))
    key = (with_qbias, with_vbias)
    if key not in _nc_cache:
        _nc_cache[key] = _make_bass(*key)
    nc = _nc_cache[key]

    encTs = [_pack_x(encoder_out[b]) for b in range(B)]  # [TKG,128,EC,512]
    in_maps = []
    for c in range(NCORES):
        b, h = divmod(c, 2)
        xTs = _pack_x(x[b, h * TQL:(h + 1) * TQL, :])
        in_maps.append({
            "xT": xTs, "encT": encTs[b],
            "wpack": wpack, "bpack": bpack, "bv": b_v,
        })

    res = bass_utils.run_bass_kernel_spmd(nc, in_maps, core_ids=list(range(NCORES)))
    kernel.last_result = res

    out = np.empty((B, TQ, D), np.float32)
    for c in range(NCORES):
        b, h = divmod(c, 2)
        pvd = np.asarray(res.results[c]["pvd"], np.float32)  # [D, NG*1024]
        rd = np.asarray(res.results[c]["rd"], np.float32)    # [NG, 2, 512]
        for g in range(NG):
            A = pvd[:, g * 1024:(g + 1) * 1024]
            A1, A2 = A[:, 0:512], A[:, 512:1024]
            r1, r2 = rd[g, 0], rd[g, 1]
            o = A1 / r1 - lam * (A2 / r2)    # [D, 512]
            q0 = h * TQL + g * 512
            out[b, q0:q0 + 512, :] = o.T
    return out


# revision 39
# speedup vs baseline: 1.2105x; 1.0099x over previous
"""Differential cross-attention head on 8 Trainium2 NeuronCores.

Sharding: data-parallel over batch (4) x sequence-parallel over Tq (2) = 8 cores.
Each core computes out[b, h*1024:(h+1)*1024, :] for (b, h) = divmod(core, 2).

Per-core math in "transposed" orientation (no on-chip transposes; host
transposes the output back):
  - qT = Wq^T @ xT            [D, 1024]
  - kT = Wk^T @ encT          [D, Tk]
  - v  = encT^T @ Wv          [Tk, D]
  - s^T = k @ q^T             [Tk, Tq] scores transposed; s1|s2 packed into one
                              [128,1024] PSUM tile via PE row-group tiling
  - e^T = exp(s^T/8)          ScalarE, PSUM->SBUF, bf16
  - A^T += v_chunk^T @ e^T    accumulated in PSUM
  - row-sums: DVE chain accumulation + ones-matmul partition reduce
Normalization out = A1/r1 - lam*A2/r2 and final transpose happen on the host.

Schedule: both q-groups interleaved per Tk chunk so ScalarE (exp, the ~33us
floor) never waits for a phase change. PSUM budget (8 banks): 2 rotating score
tiles (4) + pv0 [A1|A2 of g0] (2) + pv1 [A1 of g1] (1) + a dedicated projection
bank (1). Group-1's A2 half is accumulated later from stored exp tiles, after
the last projection releases the projection bank - that is what frees the bank
that lets k/v projections software-pipeline into the previous Tk-group's
attention stream without stalling ScalarE.

b_k is never applied: adding k-bias shifts every score of a query column by
the same constant (s[k,q] += bk . q[:,q]), and softmax over Tk is invariant
to per-column shifts. b_q does change the weights and is applied when nonzero.

DMA: all transfers have 4-8KB contiguous rows (DMA engines serve queues
packet-by-packet, so per-queue bandwidth scales with packet size), split
across the two HWDGE queues (Sync, Scalar) + GpSimd SWDGE, priority-ordered.
"""

import sys
from contextlib import ExitStack

import numpy as np

_TRN_REPO = "/opt/trn_rl_repo"
if _TRN_REPO not in sys.path:
    sys.path.insert(0, _TRN_REPO)

import ml_dtypes

import concourse.bass as bass
import concourse.tile as tile
from concourse import mybir
from concourse.bass import ds, ts

F32 = mybir.dt.float32
BF16 = mybir.dt.bfloat16

E = 1024
D = 128
B = 4
TQ = 2048
TK = 2048
NCORES = 8
TQL = B * TQ // NCORES   # 1024 query rows per core
EC = E // 128            # 8 contraction chunks for projections
NG = TQL // 512          # 2 query groups of 512
TKG = TK // 512          # 4 Tk groups
KC = TK // 128           # 16 Tk chunks
SCALE = 0.125            # 1/sqrt(64)

NP_BF16 = ml_dtypes.bfloat16


def _build(nc: bass.Bass, with_qbias: bool, with_vbias: bool):
    xT = nc.dram_tensor("xT", [NG, 128, EC, 512], BF16,
                        kind="ExternalInput").ap()
    encT = nc.dram_tensor("encT", [TKG, 128, EC, 512], BF16,
                          kind="ExternalInput").ap()
    wpack = nc.dram_tensor("wpack", [128, 3, EC, D], BF16,
                           kind="ExternalInput").ap()
    bpack = nc.dram_tensor("bpack", [128, 2], F32, kind="ExternalInput").ap()
    bv = nc.dram_tensor("bv", [D], F32, kind="ExternalInput").ap()
    pvd = nc.dram_tensor("pvd", [128, NG * 1024], BF16,
                         kind="ExternalOutput").ap()
    rd = nc.dram_tensor("rd", [NG, 2, 512], F32, kind="ExternalOutput").ap()

    Exp = mybir.ActivationFunctionType.Exp

    with tile.TileContext(nc) as tc, ExitStack() as ctx:
        const = ctx.enter_context(tc.tile_pool(name="const", bufs=1))
        xpool = ctx.enter_context(tc.tile_pool(name="xpool", bufs=1))
        encpool = ctx.enter_context(tc.tile_pool(name="encpool", bufs=1))
        proj = ctx.enter_context(tc.tile_pool(name="proj", bufs=1))
        epool = ctx.enter_context(tc.tile_pool(name="epool", bufs=6))
        g1pool = ctx.enter_context(tc.tile_pool(name="g1pool", bufs=KC))
        rpool = ctx.enter_context(tc.tile_pool(name="rpool", bufs=4))
        outp = ctx.enter_context(tc.tile_pool(name="outp", bufs=2))
        psS = ctx.enter_context(tc.tile_pool(name="psS", bufs=2, space="PSUM"))
        psPV = ctx.enter_context(tc.tile_pool(name="psPV", bufs=1, space="PSUM"))
        psP = ctx.enter_context(tc.tile_pool(name="psP", bufs=1, space="PSUM"))

        # ---- input DMAs: 8KB contiguous rows everywhere (queue bandwidth is
        # packet-proportional); big tensors partition-split [0:64]/[64:128]
        # so three queues can be loaded to match each piece's need-time ----
        w3_sb = const.tile([128, 3, EC, D], BF16, tag="w3")
        xstage = xpool.tile([128, NG, EC, 512], BF16, tag="xstage")
        enc_sb = encpool.tile([128, TKG, EC, 512], BF16, tag="enc")
        P0, P1 = slice(0, 64), slice(64, 128)

        # sync HWDGE: wq+wk then x-g0 (full width, 8KB rows)
        nc.sync.dma_start(out=w3_sb[:, 0:2], in_=wpack[:, 0:2])   # wq, wk
        if with_qbias:
            b_sb = const.tile([128, 2], F32, tag="b")
            nc.sync.dma_start(out=b_sb, in_=bpack)
        nc.sync.dma_start(out=xstage[:, 0], in_=xT[0])

        # scalar HWDGE: enc-tg0, wv, enc-tg1, x-g1, enc-tg2 (need-time order)
        nc.scalar.dma_start(out=enc_sb[:, 0], in_=encT[0])
        nc.scalar.dma_start(out=w3_sb[:, 2:3], in_=wpack[:, 2:3])  # wv
        if with_vbias:
            bv_sb = const.tile([1, D], F32, tag="bv")
            nc.scalar.dma_start(out=bv_sb,
                                in_=bv.rearrange("(o d) -> o d", o=1))
        nc.scalar.dma_start(out=enc_sb[:, 1], in_=encT[1])
        nc.scalar.dma_start(out=xstage[:, 1], in_=xT[1])
        nc.scalar.dma_start(out=enc_sb[:, 2], in_=encT[2])

        # gpsimd SWDGE: enc-tg3 (needed last; tolerates slow completion sem)
        nc.gpsimd.dma_start(out=enc_sb[:, 3], in_=encT[3])
        _ = P0, P1

        if with_vbias:
            ones_row_f32 = const.tile([1, 128], F32, tag="ones_row_f32")
            nc.vector.memset(ones_row_f32, 1.0)
        ones_col = const.tile([128, 1], BF16, tag="ones_col")
        nc.vector.memset(ones_col, 1.0)


        qT_sb = proj.tile([128, TQL], BF16, tag="qT")
        kT_sb = proj.tile([128, TK], BF16, tag="kT")
        v_sb = proj.tile([128, KC, D], BF16, tag="v")

        def qproj(g, pool):
            qp = pool.tile([128, 512] if pool is psP else [128, 1024],
                           F32, tag="ps_p" if pool is psP else "ps_s",
                           name=f"qp{g}")
            for c in range(EC):
                nc.tensor.matmul(qp[:, 0:512], lhsT=w3_sb[:, 0, c],
                                 rhs=xstage[:, g, c],
                                 start=(c == 0), stop=(c == EC - 1))
            if with_qbias:
                nc.vector.tensor_scalar_add(qT_sb[:, ts(g, 512)], qp[:, 0:512],
                                            b_sb[:, 0:1])
            else:
                nc.vector.tensor_copy(qT_sb[:, ts(g, 512)], qp[:, 0:512])

        # ---- projection steps for one tk-group, as small closures that the
        # main loop interleaves between attention units ----
        kp_box = [None]
        vp_box = [None]

        def kp_mm(tg, c0, c1):
            def step():
                if c0 == 0:
                    kp_box[0] = psP.tile([128, 512], F32, tag="ps_p",
                                         name=f"kp{tg}")
                for c in range(c0, c1):
                    nc.tensor.matmul(kp_box[0], lhsT=w3_sb[:, 1, c],
                                     rhs=enc_sb[:, tg, c],
                                     start=(c == 0), stop=(c == EC - 1))
            return step

        def kp_drain(tg):
            def step():
                # b_k is dropped: softmax over Tk is invariant to the
                # per-query-column shift it induces.
                nc.vector.tensor_copy(kT_sb[:, ts(tg, 512)], kp_box[0])
            return step

        def vp_mm(tg, t):
            def step():
                if t == 0:
                    vp_box[0] = psP.tile([128, 512], F32, tag="ps_p",
                                         name=f"vp{tg}")
                tk = tg * 4 + t
                if with_vbias:
                    nc.tensor.matmul(vp_box[0][:, ts(t, 128)],
                                     lhsT=ones_row_f32, rhs=bv_sb,
                                     start=True, stop=False,
                                     skip_group_check=True)
                for c in range(EC):
                    nc.tensor.matmul(vp_box[0][:, ts(t, 128)],
                                     lhsT=enc_sb[:, tg, c, ts(t, 128)],
                                     rhs=w3_sb[:, 2, c],
                                     start=(not with_vbias and c == 0),
                                     stop=(c == EC - 1),
                                     skip_group_check=True)
            return step

        def vp_drain(tg):
            def step():
                nc.vector.tensor_copy(
                    v_sb[:, tg * 4:(tg + 1) * 4, :]
                    .rearrange("p t d -> p (t d)"),
                    vp_box[0])
            return step

        def proj_steps(tg):
            return [kp_mm(tg, 0, 4), kp_mm(tg, 4, 8), kp_drain(tg),
                    vp_mm(tg, 0), vp_mm(tg, 1), vp_mm(tg, 2), vp_mm(tg, 3),
                    vp_drain(tg)]

        # ---- attention ----
        pv0 = [None]
        pv1 = [None]
        a2_box = [None]
        g1e = [None] * KC
        racc = [[rpool.tile([128, 1024], BF16, tag="racc", name=f"racc{g}{p}")
                 for p in range(2)] for g in range(NG)]

        def emit_pv(g, k_glob, e12):
            if g == 0:
                if k_glob == 0:
                    pv0[0] = psPV.tile([128, 1024], F32, tag="pv0", name="pv0")
                for h in range(2):
                    nc.tensor.matmul(pv0[0][:, ts(h, 512)],
                                     lhsT=v_sb[:, k_glob, :],
                                     rhs=e12[:, ts(h, 512)],
                                     start=(k_glob == 0),
                                     stop=(k_glob == KC - 1),
                                     skip_group_check=True)
            else:
                if k_glob == 0:
                    pv1[0] = psPV.tile([128, 512], F32, tag="pv1", name="pv1")
                nc.tensor.matmul(pv1[0],
                                 lhsT=v_sb[:, k_glob, :],
                                 rhs=e12[:, 0:512],
                                 start=(k_glob == 0), stop=(k_glob == KC - 1),
                                 skip_group_check=True)

        g0e = {}

        def attention_unit(g, k_glob, defer_pv=False):
            s12 = psS.tile([128, 1024], F32, tag="ps_s", name="s12")
            nc.tensor.matmul(s12[:, 0:512],
                             lhsT=kT_sb[0:64, ts(k_glob, 128)],
                             rhs=qT_sb[0:64, ts(g, 512)],
                             start=True, stop=True, tile_position=(0, 0))
            nc.tensor.matmul(s12[:, 512:1024],
                             lhsT=kT_sb[64:128, ts(k_glob, 128)],
                             rhs=qT_sb[64:128, ts(g, 512)],
                             start=True, stop=True, tile_position=(64, 0))
            if g == 0:
                e12 = epool.tile([128, 1024], BF16, tag="e", name="e12")
            else:
                e12 = g1pool.tile([128, 1024], BF16, tag="e1",
                                  name=f"e1_{k_glob}")
                g1e[k_glob] = e12
            nc.scalar.activation(e12, s12, Exp, scale=SCALE)
            if defer_pv:
                g0e[k_glob] = e12
            else:
                emit_pv(g, k_glob, e12)
            par = k_glob % 2
            if k_glob < 2:
                nc.vector.tensor_copy(racc[g][par], e12)
            else:
                nc.vector.tensor_add(racc[g][par], racc[g][par], e12)

        def pv_flush(*ks):
            def step():
                for k in ks:
                    emit_pv(0, k, g0e.pop(k))
            return step

        def a2_mm(k):
            # group-1 A2 = sum_k v^T exp(s2), from stored exp tiles, using
            # the projection bank after the last projection released it
            if k == 0:
                a2_box[0] = psP.tile([128, 512], F32, tag="ps_p", name="a2")
            nc.tensor.matmul(a2_box[0], lhsT=v_sb[:, k, :],
                             rhs=g1e[k][:, 512:1024],
                             start=(k == 0), stop=(k == KC - 1),
                             skip_group_check=True)

        def attention_tail(g):
            # both racc chains reduced directly in PSUM accumulation - no
            # DVE merge on the critical tail
            r12p = psS.tile([65, 512], F32, tag="ps_s", name="r12p")
            for p in range(2):
                nc.tensor.matmul(r12p[0:1, :], lhsT=ones_col,
                                 rhs=racc[g][p][:, 0:512],
                                 start=(p == 0), stop=(p == 1),
                                 skip_group_check=True)
                nc.tensor.matmul(r12p[64:65, :], lhsT=ones_col,
                                 rhs=racc[g][p][:, 512:1024],
                                 start=(p == 0), stop=(p == 1),
                                 skip_group_check=True)
            r_sb = outp.tile([65, 512], F32, tag="r_sb", name=f"r_sb{g}")
            nc.vector.tensor_copy(r_sb, r12p)
            nc.sync.dma_start(out=rd[g, 0], in_=r_sb[0:1, :])
            nc.sync.dma_start(out=rd[g, 1], in_=r_sb[64:65, :])
            pv_sb = outp.tile([128, 1024], BF16, tag="pv_sb", name=f"pv_sb{g}")
            if g == 0:
                nc.scalar.copy(pv_sb, pv0[0])
                nc.sync.dma_start(out=pvd[:, ds(0, 1024)], in_=pv_sb)
            else:
                # A1 half drains and ships while A2 still accumulates
                nc.scalar.copy(pv_sb[:, 0:512], pv1[0])
                nc.sync.dma_start(out=pvd[:, ds(1024, 512)],
                                  in_=pv_sb[:, 0:512])
                nc.scalar.copy(pv_sb[:, 512:1024], a2_box[0])
                nc.scalar.dma_start(out=pvd[:, ds(1536, 512)],
                                    in_=pv_sb[:, 512:1024])

        # ---- schedule: group 1 lags LAG chunks behind group 0 so its late
        # x / qT arrival never blocks the in-order PE stream; projections for
        # tk-group N spread over stream positions 4(N-1)..4(N-1)+3; A2
        # catch-up once the projection bank is released ----
        LAG = 4

        # prologue: k-proj(tg0) and qT(g0) only, then the first two attention
        # units fire (PV deferred); v-proj(tg0) + tg1's projections follow
        # while the exp stream is already running
        ps0 = proj_steps(0)
        for step in ps0[0:3]:
            step()
        qproj(0, psS)
        attention_unit(0, 0, defer_pv=True)
        attention_unit(0, 1, defer_pv=True)
        ps0[3]()
        attention_unit(0, 2, defer_pv=True)
        ps0[4]()
        attention_unit(0, 3, defer_pv=True)
        for step in ps0[5:8]:
            step()
        pv_flush(0, 1, 2, 3)()

        pos_steps = {}
        s1 = proj_steps(1)
        s1.append(lambda: qproj(1, psP))
        for step in s1[0:4]:
            step()
        pos_steps[2] = s1[4:6]
        pos_steps[3] = s1[6:9]
        s2 = proj_steps(2)
        pos_steps[4] = s2[0:2]
        pos_steps[5] = s2[2:4]
        pos_steps[6] = s2[4:6]
        pos_steps[7] = s2[6:8]
        s3 = proj_steps(3)
        pos_steps[8] = s3[0:2]
        pos_steps[9] = s3[2:4]
        pos_steps[10] = s3[4:6]
        pos_steps[11] = s3[6:8]
        pos_steps[12] = [lambda k=k: a2_mm(k) for k in (0, 1, 2)]
        pos_steps[13] = [lambda k=k: a2_mm(k) for k in (3, 4, 5)]
        pos_steps[14] = [lambda k=k: a2_mm(k) for k in (6, 7)]
        pos_steps[15] = [lambda k=k: a2_mm(k) for k in (8, 9)]

        for p in range(2, KC):
            if p >= 4:
                attention_unit(0, p)
            if p >= LAG:
                attention_unit(1, p - LAG)
            for step in pos_steps.get(p, []):
                step()

        # group-0 tail overlaps group-1's remaining units
        attention_tail(0)
        for k in range(KC - LAG, KC):
            attention_unit(1, k)
            a2_mm(k - 2)
            if k == KC - 1:
                a2_mm(k - 1)
                a2_mm(k)
        attention_tail(1)

    return nc


_nc_cache = {}


def _make_bass(with_qbias: bool, with_vbias: bool):
    from concourse import bacc

    nc = bacc.Bacc("TRN2", target_bir_lowering=False, debug=False)
    _build(nc, with_qbias, with_vbias)
    nc.compile()
    return nc


def _pack_x(a):
    """[T, E] -> [T//512, 128, EC, 512] bf16 (partition-major per group)."""
    t = a.shape[0]
    at = np.ascontiguousarray(a.T.astype(NP_BF16))       # [E, T]
    at = at.reshape(EC, 128, t // 512, 512)              # [c, p, g, 512]
    return np.ascontiguousarray(at.transpose(2, 1, 0, 3))  # [g, p, c, 512]


def _pack_w(W_q, W_k, W_v):
    """3x [E, D] -> [128, 3, EC, D] bf16."""
    w = np.stack([np.asarray(w, np.float32) for w in (W_q, W_k, W_v)])
    w = w.astype(NP_BF16).reshape(3, EC, 128, D)
    return np.ascontiguousarray(w.transpose(2, 0, 1, 3))


def kernel(x, encoder_out, W_q, b_q, W_k, b_k, W_v, b_v,
           lambda_q1, lambda_k1, lambda_q2, lambda_k2, lambda_init):
    from concourse import bass_utils

    x = np.asarray(x, np.float32)
    encoder_out = np.asarray(encoder_out, np.float32)
    wpack = _pack_w(W_q, W_k, W_v)
    bpack = np.ascontiguousarray(
        np.stack([np.asarray(b_q, np.float32),
                  np.asarray(b_k, np.float32)], axis=1))  # [128, 2]
    b_v = np.asarray(b_v, np.float32)

    lam = np.float32(
        np.exp(np.float32(np.asarray(lambda_q1, np.float32)
                          @ np.asarray(lambda_k1, np.float32)))
        - np.exp(np.float32(np.asarray(lambda_q2, np.float32)
                            @ np.asarray(lambda_k2, np.float32)))
        + np.float32(np.asarray(lambda_init, np.float32))
    )

    with_qbias = bool(np.any(np.asarray(b_q, np.float32)))
    with_vbias = bool(np.any(b_v))
    key = (with_qbias, with_vbias)
    if key not in _nc_cache:
        _nc_cache[key] = _make_bass(*key)
    nc = _nc_cache[key]

    encTs = [_pack_x(encoder_out[b]) for b in range(B)]  # [TKG,128,EC,512]
    in_maps = []
    for c in range(NCORES):
        b, h = divmod(c, 2)
        xTs = _pack_x(x[b, h * TQL:(h + 1) * TQL, :])
        in_maps.append({
            "xT": xTs, "encT": encTs[b],
            "wpack": wpack, "bpack": bpack, "bv": b_v,
        })

    res = bass_utils.run_bass_kernel_spmd(nc, in_maps, core_ids=list(range(NCORES)))
    kernel.last_result = res

    out = np.empty((B, TQ, D), np.float32)
    for c in range(NCORES):
        b, h = divmod(c, 2)
        pvd = np.asarray(res.results[c]["pvd"], np.float32)  # [D, NG*1024]
        rd = np.asarray(res.results[c]["rd"], np.float32)    # [NG, 2, 512]
        for g in range(NG):
            A = pvd[:, g * 1024:(g + 1) * 1024]
            A1, A2 = A[:, 0:512], A[:, 512:1024]
            r1, r2 = rd[g, 0], rd[g, 1]
            o = A1 / r1 - lam * (A2 / r2)    # [D, 512]
            q0 = h * TQL + g * 512
            out[b, q0:q0 + 512, :] = o.T
    return out
